# revision 17
# baseline (speedup 1.0000x reference)
"""Trainium2 Bass kernel for per-node multi-head attention (GNN message passing).

Math (per node n):
  q = (h @ Wq + bq).reshape(4, 64);  k, v likewise
  attn = softmax((q @ k.T) / 8, axis=-1)      # [4, 4], across heads
  out  = (attn @ v).reshape(256)

Strategy: pure data parallel over 8 cores (62500 nodes each), node-on-partition
layout (128 nodes per tile).  The host pre-transposes h to hT[256, N] in bf16,
so each tile's hT chunk IS the matmul's stationary operand (lhsT) directly --
no on-chip transpose, no PSUM round-trip for h.  The q/k biases are folded into
a host-precomputed 16-column C matrix appended to the packed weights
(logits = tree(q0*k0) + h @ C + const4), which removes the 768-column bias
matmuls from the PE (PE's HAM stays cold at 1.2 GHz on this part, so streamed
columns are expensive).  Wv is (d,g)-column-reordered so the AV products read
contiguously.

Tiles run in groups of QUAD: one input DMA per group (contiguous 1.25 KB
lines), per-tile PE frontend (2 data matmuls per K-chunk + one 272-wide
ones-row matmul adding bv|const4 in a single PSUM accumulation group per bank
-- start=True clears at BANK granularity, so v and C must share one group),
2 ACT PSUM->SBUF copies into dedicated q / k|v|C slabs, and a batched
per-group backend drained from a FIFO at ~3 ops per tile.

Engine assignment (HW-measured): all backend tensor ops on DVE, where every
access pattern is built to merge to <=3 free dims so the bf16 2x mode holds
(products, tree slices, and rows-of-2 adds all measure ~0.55 ns/elem); only
the tiny final tree add runs on GpSimd.  GpSimd is otherwise kept idle: it
measures 2-4 ns/elem and, when loaded, degrades concurrent DVE ops from 2x
to 1x (SBUF port contention) -- moving any bulk stage there is a net loss.

HW (NTFF, 8 cores): 1.248 ms span, DVE 93% busy (the bottleneck: ~11.9 us
per 5-tile group of products + trees + softmax smalls), vs 1.55 ms for the
previous PE-transpose + broadcast-heavy schedule.
"""

import sys

sys.path.insert(0, "/opt/trn_rl_repo")

import numpy as np
import ml_dtypes

import concourse.bass as bass
import concourse.bacc as bacc
import concourse.tile as tile
from concourse import mybir
from concourse.bass_utils import run_bass_kernel_spmd

N_CORES = 8
N_TOTAL = 500000
SHARD = N_TOTAL // N_CORES  # 62500
IN = 256
OUT = 256
NH = 4
HD = 64
P = 128
QUAD = 5  # tiles per batched backend group

BF16 = mybir.dt.bfloat16
F32 = mybir.dt.float32
ALU = mybir.AluOpType
AX = mybir.AxisListType
ACTF = mybir.ActivationFunctionType


def build_program(shard_rows: int, compile: bool = True) -> bass.Bass:
    nc = bacc.Bacc()

    hT_ext = nc.declare_dram_parameter("hT", [IN, shard_rows], BF16, isOutput=False)
    w_ext = nc.declare_dram_parameter("w", [IN, 3 * OUT + 16], BF16, isOutput=False)
    bias_ext = nc.declare_dram_parameter("bias", [1, OUT + 16], BF16, isOutput=False)
    out_ext = nc.declare_dram_parameter("out", [shard_rows, OUT], F32, isOutput=True)

    n_full, tail = divmod(shard_rows, P)
    tiles = [(i, P) for i in range(n_full)]
    if tail:
        tiles.append((n_full, tail))

    with tile.TileContext(nc) as tc:
        with (
            tc.tile_pool(name="consts", bufs=1) as consts,
            tc.tile_pool(name="io", bufs=4) as io,
            tc.tile_pool(name="qkv", bufs=3) as qkv,
            tc.tile_pool(name="mid", bufs=3) as mid,
            tc.tile_pool(name="outp", bufs=3) as outp,
            tc.tile_pool(name="ps", bufs=4, space="PSUM") as ps,
        ):
            # weights: [128, 2, 784] (2 K-chunks, [Wq'|Wk|Wv_r|C]); bias row; ones
            w_sb = consts.tile([P, 2, 3 * OUT + 16], BF16)
            for c in range(2):
                nc.scalar.dma_start(out=w_sb[:, c], in_=w_ext[c * P : (c + 1) * P, :])
            bias_sb = consts.tile([1, OUT + 16], BF16)
            nc.scalar.dma_start(out=bias_sb, in_=bias_ext[:, :])
            ones_sb = consts.tile([1, P], BF16)
            nc.vector.memset(ones_sb, 1.0)

            from collections import deque

            groups = []
            backlog = deque()

            def new_group(ragged, r0, gn):
                rec = {
                    "members": [],
                    "ragged": ragged,
                    "r0": r0,
                    "gn": gn,  # nodes in this group
                    "hTs": io.tile([P, 2, QUAD * P], BF16, tag="hTs", name="hTs"),
                    "qs": qkv.tile([P, QUAD, OUT], BF16, tag="qs", name="qs"),
                    "kvc": qkv.tile([P, QUAD, 2 * OUT + 16], BF16, tag="kvc", name="kvc"),
                    "p1s": mid.tile([P, QUAD, NH * NH, HD], BF16, tag="p1s", name="p1s"),
                    "t1": mid.tile([P, QUAD, NH * NH, 32], BF16, tag="t1", name="t1"),
                    "t2": mid.tile([P, QUAD, NH * NH, 16], BF16, tag="t2", name="t2"),
                    "t3": mid.tile([P, QUAD, NH * NH, 8], BF16, tag="t3", name="t3"),
                    "t4": mid.tile([P, QUAD, NH * NH, 4], BF16, tag="t4", name="t4"),
                    "t5": mid.tile([P, QUAD, NH * NH, 2], BF16, tag="t5", name="t5"),
                    "logits0": mid.tile([P, QUAD, NH * NH], F32, tag="logits0", name="logits0"),
                    "logits": mid.tile([P, QUAD, NH * NH], F32, tag="logits", name="logits"),
                    "ex": mid.tile([P, QUAD, NH, NH], BF16, tag="ex", name="ex"),
                    "den": mid.tile([P, QUAD * NH], F32, tag="den", name="den"),
                    "rcp": mid.tile([P, QUAD * NH], F32, tag="rcp", name="rcp"),
                    "attn": mid.tile([P, QUAD, NH, NH], BF16, tag="attn", name="attn"),
                    "p2s": outp.tile([P, QUAD, NH, HD, NH], BF16, tag="p2s", name="p2s"),
                    "av1": outp.tile([P, QUAD, NH, HD, 2], BF16, tag="av1", name="av1"),
                    "oslab": outp.tile([P, QUAD, OUT], F32, tag="oslab", name="oslab"),
                }
                if ragged:
                    nc.vector.memset(rec["qs"], 0.0)
                    nc.vector.memset(rec["kvc"], 0.0)
                else:
                    for c in range(2):
                        nc.sync.dma_start(
                            out=rec["hTs"][:, c],
                            in_=hT_ext[c * P : (c + 1) * P, r0 : r0 + QUAD * P],
                        )
                groups.append(rec)
                return rec

            def emit_p1(rec):
                qb = (
                    rec["qs"]
                    .rearrange("p q (h one d) -> p q h one d", h=NH, one=1)
                    .to_broadcast([P, QUAD, NH, NH, HD])
                )
                kb = (
                    rec["kvc"][:, :, 0:256]
                    .rearrange("p q (one g d) -> p q one g d", one=1, g=NH)
                    .to_broadcast([P, QUAD, NH, NH, HD])
                )
                nc.vector.tensor_tensor(
                    out=rec["p1s"].rearrange("p q (h g) d -> p q h g d", h=NH),
                    in0=qb,
                    in1=kb,
                    op=ALU.mult,
                )

            def emit_tree(rec, level):
                src = (rec["p1s"], rec["t1"], rec["t2"], rec["t3"], rec["t4"])[
                    level - 1
                ]
                dst = (rec["t1"], rec["t2"], rec["t3"], rec["t4"], rec["t5"])[
                    level - 1
                ]
                w = 64 >> level
                eng = nc.vector
                eng.tensor_tensor(
                    out=dst,
                    in0=src[:, :, :, 0:w],
                    in1=src[:, :, :, w : 2 * w],
                    op=ALU.add,
                )

            def emit_treef(rec):
                t5 = rec["t5"]
                nc.gpsimd.tensor_tensor(
                    out=rec["logits0"].rearrange("p q (f one) -> p q f one", one=1),
                    in0=t5[:, :, :, 0:1],
                    in1=t5[:, :, :, 1:2],
                    op=ALU.add,
                )

            def emit_treec(rec):
                # logits = tree + (h @ C + const4), the bias cross-terms
                nc.vector.tensor_tensor(
                    out=rec["logits"],
                    in0=rec["logits0"],
                    in1=rec["kvc"][:, :, 512:528],
                    op=ALU.add,
                )

            def emit_exp(rec):
                nc.scalar.activation(
                    out=rec["ex"].rearrange("p q h g -> p (q h g)"),
                    in_=rec["logits"].rearrange("p q f -> p (q f)"),
                    func=ACTF.Exp,
                )

            def emit_den(rec):
                nc.vector.tensor_reduce(
                    out=rec["den"],
                    in_=rec["ex"].rearrange("p q h g -> p (q h) g"),
                    axis=AX.X,
                    op=ALU.add,
                )

            def emit_rcp(rec):
                nc.vector.reciprocal_approx_fast(out=rec["rcp"], in_=rec["den"])

            def emit_attn(rec):
                nc.vector.tensor_tensor(
                    out=rec["attn"],
                    in0=rec["ex"],
                    in1=rec["rcp"]
                    .rearrange("p (q h one) -> p q h one", q=QUAD, one=1)
                    .to_broadcast([P, QUAD, NH, NH]),
                    op=ALU.mult,
                )

            def emit_p2(rec):
                ab = (
                    rec["attn"]
                    .rearrange("p q h (one g) -> p q h one g", one=1)
                    .to_broadcast([P, QUAD, NH, HD, NH])
                )
                vb = (
                    rec["kvc"][:, :, 256:512]
                    .rearrange("p q (one d g) -> p q one d g", one=1, d=HD)
                    .to_broadcast([P, QUAD, NH, HD, NH])
                )
                nc.vector.tensor_tensor(out=rec["p2s"], in0=ab, in1=vb, op=ALU.mult)

            def emit_av1(rec):
                p2 = rec["p2s"]
                nc.vector.tensor_tensor(
                    out=rec["av1"],
                    in0=p2[:, :, :, :, 0:2],
                    in1=p2[:, :, :, :, 2:4],
                    op=ALU.add,
                )

            def emit_avf(rec):
                av1 = rec["av1"]
                nc.vector.tensor_tensor(
                    out=rec["oslab"].rearrange("p q (h d) -> p q h d", h=NH),
                    in0=av1[:, :, :, :, 0],
                    in1=av1[:, :, :, :, 1],
                    op=ALU.add,
                )

            def emit_outdma(rec):
                if not rec["ragged"]:
                    nc.sync.dma_start(
                        out=out_ext[rec["r0"] : rec["r0"] + QUAD * P, :].rearrange(
                            "(t p) f -> p t f", p=P
                        ),
                        in_=rec["oslab"],
                    )
                else:
                    for t, (i, p) in enumerate(rec["members"]):
                        nc.sync.dma_start(
                            out=out_ext[i * P : i * P + p, :],
                            in_=rec["oslab"][:p, t],
                        )

            BACKEND = [
                emit_p1,
                lambda g: emit_tree(g, 1),
                lambda g: emit_tree(g, 2),
                lambda g: emit_tree(g, 3),
                lambda g: emit_tree(g, 4),
                lambda g: emit_tree(g, 5),
                emit_treef,
                emit_treec,
                emit_exp,
                emit_den,
                emit_rcp,
                emit_attn,
                emit_p2,
                emit_av1,
                emit_avf,
                emit_outdma,
            ]

            for idx, (i, p) in enumerate(tiles):
                t = idx % QUAD
                if t == 0:
                    n_rem = len(tiles) - idx
                    ragged = n_rem < QUAD or (n_rem == QUAD and tiles[-1][1] < P)
                    cur = new_group(ragged, i * P, min(n_rem, QUAD))
                r0 = i * P
                if cur["ragged"]:
                    for c in range(2):
                        nc.sync.dma_start(
                            out=cur["hTs"][:, c, t * P : t * P + p],
                            in_=hT_ext[c * P : (c + 1) * P, r0 : r0 + p],
                        )

                # ---- per-tile frontend: PE matmuls + ACT copies ----
                # regions: q+k [0:512] (no bias -- folded into C), v [512:768]
                # (+bv via ones-MM), C [768:784] (+const4 via ones-MM).
                qkv_ps = ps.tile([p, 3 * OUT + 16], F32, tag="qkv_ps", name="qkv_ps")
                for c in range(2):
                    lhs = cur["hTs"][:, c, t * P : t * P + p]
                    nc.tensor.matmul(
                        out=qkv_ps[:, 0:512],
                        lhsT=lhs,
                        rhs=w_sb[:, c, 0:512],
                        start=(c == 0),
                        stop=(c == 1),
                    )
                    nc.tensor.matmul(
                        out=qkv_ps[:, 512:784],
                        lhsT=lhs,
                        rhs=w_sb[:, c, 512:784],
                        start=(c == 0),
                        stop=False,
                    )
                nc.tensor.matmul(
                    out=qkv_ps[:, 512:784],
                    lhsT=ones_sb[:, :p],
                    rhs=bias_sb[:, 0:272],
                    start=False,
                    stop=True,
                )

                nc.scalar.copy(out=cur["qs"][:p, t], in_=qkv_ps[:, 0:256])
                nc.scalar.copy(out=cur["kvc"][:p, t], in_=qkv_ps[:, 256:784])
                cur["members"].append((i, p))

                # software pipelining: drain finished groups' backend ops
                pops = 4 if len(backlog) > len(BACKEND) else 3
                for _ in range(pops):
                    if backlog:
                        backlog.popleft()()
                if t == QUAD - 1 or idx == len(tiles) - 1:
                    g = cur
                    backlog.extend([lambda g=g, f=f: f(g) for f in BACKEND])

            while backlog:
                backlog.popleft()()

    if compile:
        nc.compile()
    return nc


def prepare_weights(Wq, bq, Wk, bk, Wv, bv):
    """Host-side transforms: fold softmax scale into q, reorder Wv/bv to
    (d, g) column order, fold the q/k biases into a 16-column C matrix
    (logits = tree(q0 (x) k0) + h @ C + const4), pack [Wq'|Wk|Wv_r|C] into
    one [256, 784] bf16 matrix and [bv_r | const4] into a [1, 272] row."""
    scale = 1.0 / np.sqrt(np.float32(HD))
    bf = ml_dtypes.bfloat16
    wq = np.asarray(Wq, np.float32) * scale
    bq_s = np.asarray(bq, np.float32) * scale
    wk = np.asarray(Wk, np.float32)
    bk_ = np.asarray(bk, np.float32)
    cols = np.arange(OUT)
    perm = (cols % HD) * NH + cols // HD  # old col (g*64+d) -> new col (d*4+g)
    wv_r = np.empty((IN, OUT), np.float32)
    wv_r[:, perm] = np.asarray(Wv, np.float32)
    bv_r = np.empty((OUT,), np.float32)
    bv_r[perm] = np.asarray(bv, np.float32)
    # C[:, h*4+g] = Wq'[:, h-block] @ bk[g-block] + Wk[:, g-block] @ bq'[h-block]
    C = np.zeros((IN, NH * NH), np.float32)
    const4 = np.zeros((NH * NH,), np.float32)
    for h in range(NH):
        for g in range(NH):
            C[:, h * NH + g] = (
                wq[:, h * HD : (h + 1) * HD] @ bk_[g * HD : (g + 1) * HD]
                + wk[:, g * HD : (g + 1) * HD] @ bq_s[h * HD : (h + 1) * HD]
            )
            const4[h * NH + g] = bq_s[h * HD : (h + 1) * HD] @ bk_[g * HD : (g + 1) * HD]
    w = np.concatenate([wq, wk, wv_r, C], axis=1).astype(bf)
    bias = np.concatenate([bv_r, const4]).reshape(1, OUT + 16).astype(bf)
    return w, bias


_PROGRAM_CACHE = {}


def _get_program(rows):
    if rows not in _PROGRAM_CACHE:
        _PROGRAM_CACHE[rows] = build_program(rows)
    return _PROGRAM_CACHE[rows]


def kernel(h, Wk, bk, Wq, bq, Wv, bv):
    h = np.asarray(h, dtype=np.float32)
    w, bias = prepare_weights(Wq, bq, Wk, bk, Wv, bv)
    hT = np.ascontiguousarray(h.T).astype(ml_dtypes.bfloat16)

    nc = _get_program(SHARD)
    in_maps = []
    for i in range(N_CORES):
        in_maps.append(
            {
                "hT": np.ascontiguousarray(hT[:, i * SHARD : (i + 1) * SHARD]),
                "w": w,
                "bias": bias,
            }
        )
    # Rare transient device flakes (~1 in 6 runs observed) can corrupt an
    # execution; the program itself is deterministic, so verify and retry.
    for attempt in range(3):
        res = run_bass_kernel_spmd(nc, in_maps, core_ids=list(range(N_CORES)))
        out = np.concatenate(
            [res.results[i]["out"] for i in range(N_CORES)], axis=0
        )
        if np.isfinite(out).all():
            return out
    return out


# revision 18
# speedup vs baseline: 1.0208x; 1.0208x over previous
"""Trainium2 Bass kernel for per-node multi-head attention (GNN message passing).

Math (per node n):
  q = (h @ Wq + bq).reshape(4, 64);  k, v likewise
  attn = softmax((q @ k.T) / 8, axis=-1)      # [4, 4], across heads
  out  = (attn @ v).reshape(256)

Strategy: pure data parallel over 8 cores (62500 nodes each), node-on-partition
layout (128 nodes per tile).  The host pre-transposes h to hT[256, N] in bf16,
so each tile's hT chunk IS the matmul's stationary operand (lhsT) directly --
no on-chip transpose, no PSUM round-trip for h.  The q/k biases are folded into
a host-precomputed 16-column C matrix appended to the packed weights
(logits = tree(q0*k0) + h @ C + const4), which removes the 768-column bias
matmuls from the PE (PE's HAM stays cold at 1.2 GHz on this part, so streamed
columns are expensive).  Wv is (d,g)-column-reordered so the AV products read
contiguously.

Tiles run in groups of QUAD: one input DMA per group (contiguous 1.25 KB
lines), per-tile PE frontend (2 data matmuls per K-chunk + one 272-wide
ones-row matmul adding bv|const4 in a single PSUM accumulation group per bank
-- start=True clears at BANK granularity, so v and C must share one group),
2 ACT PSUM->SBUF copies into dedicated q / k|v|C slabs, and a batched
per-group backend drained from a FIFO at ~3 ops per tile.

Engine assignment (HW-measured): all backend tensor ops on DVE, where every
access pattern is built to merge to <=3 free dims so the bf16 2x mode holds
(products, tree slices, and rows-of-2 adds all measure ~0.55 ns/elem); only
the tiny final tree add runs on GpSimd.  GpSimd is otherwise kept idle: it
measures 2-4 ns/elem and, when loaded, degrades concurrent DVE ops from 2x
to 1x (SBUF port contention) -- moving any bulk stage there is a net loss.

HW (NTFF, 8 cores): 1.248 ms span, DVE 93% busy (the bottleneck: ~11.9 us
per 5-tile group of products + trees + softmax smalls), vs 1.55 ms for the
previous PE-transpose + broadcast-heavy schedule.
"""

import sys

sys.path.insert(0, "/opt/trn_rl_repo")

import numpy as np
import ml_dtypes

import concourse.bass as bass
import concourse.bacc as bacc
import concourse.tile as tile
from concourse import mybir
from concourse.bass_utils import run_bass_kernel_spmd

N_CORES = 8
N_TOTAL = 500000
SHARD = N_TOTAL // N_CORES  # 62500
IN = 256
OUT = 256
NH = 4
HD = 64
P = 128
QUAD = 5  # tiles per batched backend group

BF16 = mybir.dt.bfloat16
F32 = mybir.dt.float32
ALU = mybir.AluOpType
AX = mybir.AxisListType
ACTF = mybir.ActivationFunctionType


def build_program(shard_rows: int, compile: bool = True) -> bass.Bass:
    nc = bacc.Bacc()

    hT_ext = nc.declare_dram_parameter("hT", [IN, shard_rows], BF16, isOutput=False)
    w_ext = nc.declare_dram_parameter("w", [IN, 3 * OUT + 16], BF16, isOutput=False)
    bias_ext = nc.declare_dram_parameter("bias", [1, OUT + 16], BF16, isOutput=False)
    out_ext = nc.declare_dram_parameter("out", [shard_rows, OUT], F32, isOutput=True)

    n_full, tail = divmod(shard_rows, P)
    tiles = [(i, P) for i in range(n_full)]
    if tail:
        tiles.append((n_full, tail))

    with tile.TileContext(nc) as tc:
        with (
            tc.tile_pool(name="consts", bufs=1) as consts,
            tc.tile_pool(name="io", bufs=4) as io,
            tc.tile_pool(name="qkv", bufs=3) as qkv,
            tc.tile_pool(name="mid", bufs=3) as mid,
            tc.tile_pool(name="outp", bufs=3) as outp,
            tc.tile_pool(name="ps", bufs=4, space="PSUM") as ps,
        ):
            # weights: [128, 2, 784] (2 K-chunks, [Wq'|Wk|Wv_r|C]); bias row; ones
            w_sb = consts.tile([P, 2, 3 * OUT + 16], BF16)
            for c in range(2):
                nc.scalar.dma_start(out=w_sb[:, c], in_=w_ext[c * P : (c + 1) * P, :])
            bias_sb = consts.tile([1, OUT + 16], BF16)
            nc.scalar.dma_start(out=bias_sb, in_=bias_ext[:, :])
            ones_sb = consts.tile([1, P], BF16)
            nc.vector.memset(ones_sb, 1.0)

            from collections import deque

            groups = []
            backlog = deque()

            def new_group(ragged, r0, gn):
                rec = {
                    "members": [],
                    "ragged": ragged,
                    "r0": r0,
                    "gn": gn,  # nodes in this group
                    "hTs": io.tile([P, 2, QUAD * P], BF16, tag="hTs", name="hTs"),
                    "qs": qkv.tile([P, QUAD, OUT], BF16, tag="qs", name="qs"),
                    "kvc": qkv.tile([P, QUAD, 2 * OUT + 16], BF16, tag="kvc", name="kvc"),
                    "p1s": mid.tile([P, QUAD, NH * NH, HD], BF16, tag="p1s", name="p1s"),
                    "t1": mid.tile([P, QUAD, NH * NH, 32], BF16, tag="t1", name="t1"),
                    "t2": mid.tile([P, QUAD, NH * NH, 16], BF16, tag="t2", name="t2"),
                    "t3": mid.tile([P, QUAD, NH * NH, 8], BF16, tag="t3", name="t3"),
                    "t4": mid.tile([P, QUAD, NH * NH, 4], BF16, tag="t4", name="t4"),
                    "t5": mid.tile([P, QUAD, NH * NH, 2], BF16, tag="t5", name="t5"),
                    "logits0": mid.tile([P, QUAD, NH * NH], F32, tag="logits0", name="logits0"),
                    "logits": mid.tile([P, QUAD, NH * NH], F32, tag="logits", name="logits"),
                    "ex": mid.tile([P, QUAD, NH, NH], BF16, tag="ex", name="ex"),
                    "den": mid.tile([P, QUAD * NH], F32, tag="den", name="den"),
                    "rcp": mid.tile([P, QUAD * NH], F32, tag="rcp", name="rcp"),
                    "attn": mid.tile([P, QUAD, NH, NH], BF16, tag="attn", name="attn"),
                    "p2s": outp.tile([P, QUAD, NH, HD, NH], BF16, tag="p2s", name="p2s"),
                    "av1": outp.tile([P, QUAD, NH, HD, 2], BF16, tag="av1", name="av1"),
                    "oslab": outp.tile([P, QUAD, OUT], F32, tag="oslab", name="oslab"),
                }
                if ragged:
                    nc.vector.memset(rec["qs"], 0.0)
                    nc.vector.memset(rec["kvc"], 0.0)
                else:
                    for c in range(2):
                        nc.sync.dma_start(
                            out=rec["hTs"][:, c],
                            in_=hT_ext[c * P : (c + 1) * P, r0 : r0 + QUAD * P],
                        )
                groups.append(rec)
                return rec

            def emit_p1(rec):
                qb = (
                    rec["qs"]
                    .rearrange("p q (h one d) -> p q h one d", h=NH, one=1)
                    .to_broadcast([P, QUAD, NH, NH, HD])
                )
                kb = (
                    rec["kvc"][:, :, 0:256]
                    .rearrange("p q (one g d) -> p q one g d", one=1, g=NH)
                    .to_broadcast([P, QUAD, NH, NH, HD])
                )
                nc.vector.tensor_tensor(
                    out=rec["p1s"].rearrange("p q (h g) d -> p q h g d", h=NH),
                    in0=qb,
                    in1=kb,
                    op=ALU.mult,
                )

            def emit_tree(rec, level):
                src = (rec["p1s"], rec["t1"], rec["t2"], rec["t3"], rec["t4"])[
                    level - 1
                ]
                dst = (rec["t1"], rec["t2"], rec["t3"], rec["t4"], rec["t5"])[
                    level - 1
                ]
                w = 64 >> level
                eng = nc.vector
                eng.tensor_tensor(
                    out=dst,
                    in0=src[:, :, :, 0:w],
                    in1=src[:, :, :, w : 2 * w],
                    op=ALU.add,
                )

            def emit_treef(rec):
                t5 = rec["t5"]
                nc.gpsimd.tensor_tensor(
                    out=rec["logits0"].rearrange("p q (f one) -> p q f one", one=1),
                    in0=t5[:, :, :, 0:1],
                    in1=t5[:, :, :, 1:2],
                    op=ALU.add,
                )

            def emit_treec(rec):
                # logits = tree + (h @ C + const4), the bias cross-terms
                nc.vector.tensor_tensor(
                    out=rec["logits"],
                    in0=rec["logits0"],
                    in1=rec["kvc"][:, :, 512:528],
                    op=ALU.add,
                )

            def emit_exp(rec):
                nc.scalar.activation(
                    out=rec["ex"].rearrange("p q h g -> p (q h g)"),
                    in_=rec["logits"].rearrange("p q f -> p (q f)"),
                    func=ACTF.Exp,
                )

            def emit_den(rec):
                nc.vector.tensor_reduce(
                    out=rec["den"],
                    in_=rec["ex"].rearrange("p q h g -> p (q h) g"),
                    axis=AX.X,
                    op=ALU.add,
                )

            def emit_rcp(rec):
                nc.vector.reciprocal_approx_fast(out=rec["rcp"], in_=rec["den"])

            def emit_attn(rec):
                nc.vector.tensor_tensor(
                    out=rec["attn"],
                    in0=rec["ex"],
                    in1=rec["rcp"]
                    .rearrange("p (q h one) -> p q h one", q=QUAD, one=1)
                    .to_broadcast([P, QUAD, NH, NH]),
                    op=ALU.mult,
                )

            def emit_p2(rec):
                ab = (
                    rec["attn"]
                    .rearrange("p q h (one g) -> p q h one g", one=1)
                    .to_broadcast([P, QUAD, NH, HD, NH])
                )
                vb = (
                    rec["kvc"][:, :, 256:512]
                    .rearrange("p q (one d g) -> p q one d g", one=1, d=HD)
                    .to_broadcast([P, QUAD, NH, HD, NH])
                )
                nc.vector.tensor_tensor(out=rec["p2s"], in0=ab, in1=vb, op=ALU.mult)

            def emit_av1(rec):
                p2 = rec["p2s"]
                nc.vector.tensor_tensor(
                    out=rec["av1"],
                    in0=p2[:, :, :, :, 0:2],
                    in1=p2[:, :, :, :, 2:4],
                    op=ALU.add,
                )

            def emit_avf(rec):
                av1 = rec["av1"]
                nc.vector.tensor_tensor(
                    out=rec["oslab"].rearrange("p q (h d) -> p q h d", h=NH),
                    in0=av1[:, :, :, :, 0],
                    in1=av1[:, :, :, :, 1],
                    op=ALU.add,
                )

            def emit_outdma(rec):
                if not rec["ragged"]:
                    nc.sync.dma_start(
                        out=out_ext[rec["r0"] : rec["r0"] + QUAD * P, :].rearrange(
                            "(t p) f -> p t f", p=P
                        ),
                        in_=rec["oslab"],
                    )
                else:
                    for t, (i, p) in enumerate(rec["members"]):
                        nc.sync.dma_start(
                            out=out_ext[i * P : i * P + p, :],
                            in_=rec["oslab"][:p, t],
                        )

            # Backend in two phases: PHASE1 ends at the gp/ACT round-trip
            # (treef -> treec -> exp); PHASE2 (softmax-dependent half) is
            # drained one group later so DVE chews on group g+1's products
            # and tree while group g's treef/exp cross-engine chain settles
            # (the DVE queue is strict FIFO -- a waiting op blocks ready ones).
            PHASE1 = [
                emit_p1,
                lambda g: emit_tree(g, 1),
                lambda g: emit_tree(g, 2),
                lambda g: emit_tree(g, 3),
                lambda g: emit_tree(g, 4),
                lambda g: emit_tree(g, 5),
                emit_treef,
                emit_treec,
                emit_exp,
            ]
            PHASE2 = [
                emit_den,
                emit_rcp,
                emit_attn,
                emit_p2,
                emit_av1,
                emit_avf,
                emit_outdma,
            ]
            BACKEND = PHASE1 + PHASE2

            for idx, (i, p) in enumerate(tiles):
                t = idx % QUAD
                if t == 0:
                    n_rem = len(tiles) - idx
                    ragged = n_rem < QUAD or (n_rem == QUAD and tiles[-1][1] < P)
                    cur = new_group(ragged, i * P, min(n_rem, QUAD))
                r0 = i * P
                if cur["ragged"]:
                    for c in range(2):
                        nc.sync.dma_start(
                            out=cur["hTs"][:, c, t * P : t * P + p],
                            in_=hT_ext[c * P : (c + 1) * P, r0 : r0 + p],
                        )

                # ---- per-tile frontend: PE matmuls + ACT copies ----
                # regions: q+k [0:512] (no bias -- folded into C), v [512:768]
                # (+bv via ones-MM), C [768:784] (+const4 via ones-MM).
                qkv_ps = ps.tile([p, 3 * OUT + 16], F32, tag="qkv_ps", name="qkv_ps")
                for c in range(2):
                    lhs = cur["hTs"][:, c, t * P : t * P + p]
                    nc.tensor.matmul(
                        out=qkv_ps[:, 0:512],
                        lhsT=lhs,
                        rhs=w_sb[:, c, 0:512],
                        start=(c == 0),
                        stop=(c == 1),
                    )
                    nc.tensor.matmul(
                        out=qkv_ps[:, 512:784],
                        lhsT=lhs,
                        rhs=w_sb[:, c, 512:784],
                        start=(c == 0),
                        stop=False,
                    )
                nc.tensor.matmul(
                    out=qkv_ps[:, 512:784],
                    lhsT=ones_sb[:, :p],
                    rhs=bias_sb[:, 0:272],
                    start=False,
                    stop=True,
                )

                nc.scalar.copy(out=cur["qs"][:p, t], in_=qkv_ps[:, 0:256])
                nc.scalar.copy(out=cur["kvc"][:p, t], in_=qkv_ps[:, 256:784])
                cur["members"].append((i, p))

                # software pipelining: drain finished groups' backend ops
                pops = 4 if len(backlog) > len(BACKEND) else 3
                for _ in range(pops):
                    if backlog:
                        backlog.popleft()()
                if t == QUAD - 1 or idx == len(tiles) - 1:
                    g = cur
                    backlog.extend([lambda g=g, f=f: f(g) for f in PHASE1])
                    if len(groups) >= 2:
                        gp_ = groups[-2]
                        backlog.extend([lambda g=gp_, f=f: f(g) for f in PHASE2])

            while backlog:
                backlog.popleft()()
            for f in PHASE2:
                f(groups[-1])

    if compile:
        nc.compile()
    return nc


def prepare_weights(Wq, bq, Wk, bk, Wv, bv):
    """Host-side transforms: fold softmax scale into q, reorder Wv/bv to
    (d, g) column order, fold the q/k biases into a 16-column C matrix
    (logits = tree(q0 (x) k0) + h @ C + const4), pack [Wq'|Wk|Wv_r|C] into
    one [256, 784] bf16 matrix and [bv_r | const4] into a [1, 272] row."""
    scale = 1.0 / np.sqrt(np.float32(HD))
    bf = ml_dtypes.bfloat16
    wq = np.asarray(Wq, np.float32) * scale
    bq_s = np.asarray(bq, np.float32) * scale
    wk = np.asarray(Wk, np.float32)
    bk_ = np.asarray(bk, np.float32)
    cols = np.arange(OUT)
    perm = (cols % HD) * NH + cols // HD  # old col (g*64+d) -> new col (d*4+g)
    wv_r = np.empty((IN, OUT), np.float32)
    wv_r[:, perm] = np.asarray(Wv, np.float32)
    bv_r = np.empty((OUT,), np.float32)
    bv_r[perm] = np.asarray(bv, np.float32)
    # C[:, h*4+g] = Wq'[:, h-block] @ bk[g-block] + Wk[:, g-block] @ bq'[h-block]
    C = np.zeros((IN, NH * NH), np.float32)
    const4 = np.zeros((NH * NH,), np.float32)
    for h in range(NH):
        for g in range(NH):
            C[:, h * NH + g] = (
                wq[:, h * HD : (h + 1) * HD] @ bk_[g * HD : (g + 1) * HD]
                + wk[:, g * HD : (g + 1) * HD] @ bq_s[h * HD : (h + 1) * HD]
            )
            const4[h * NH + g] = bq_s[h * HD : (h + 1) * HD] @ bk_[g * HD : (g + 1) * HD]
    w = np.concatenate([wq, wk, wv_r, C], axis=1).astype(bf)
    bias = np.concatenate([bv_r, const4]).reshape(1, OUT + 16).astype(bf)
    return w, bias


_PROGRAM_CACHE = {}


def _get_program(rows):
    if rows not in _PROGRAM_CACHE:
        _PROGRAM_CACHE[rows] = build_program(rows)
    return _PROGRAM_CACHE[rows]


def kernel(h, Wk, bk, Wq, bq, Wv, bv):
    h = np.asarray(h, dtype=np.float32)
    w, bias = prepare_weights(Wq, bq, Wk, bk, Wv, bv)
    hT = np.ascontiguousarray(h.T).astype(ml_dtypes.bfloat16)

    nc = _get_program(SHARD)
    in_maps = []
    for i in range(N_CORES):
        in_maps.append(
            {
                "hT": np.ascontiguousarray(hT[:, i * SHARD : (i + 1) * SHARD]),
                "w": w,
                "bias": bias,
            }
        )
    # Rare transient device flakes (~1 in 6 runs observed) can corrupt an
    # execution; the program itself is deterministic, so verify and retry.
    for attempt in range(3):
        res = run_bass_kernel_spmd(nc, in_maps, core_ids=list(range(N_CORES)))
        out = np.concatenate(
            [res.results[i]["out"] for i in range(N_CORES)], axis=0
        )
        if np.isfinite(out).all():
            return out
    return out
